# revision 30
# baseline (speedup 1.0000x reference)
"""ConditionalAttentionLayer (gnn_message_passing) Trainium2 kernel.

Sharding: one attention mechanism (head) per NeuronCore, 8 cores.
Each core computes its head's projection h_m = FiLM(x @ W_m), GAT scores,
masked softmax and out_m = attn_m @ h_m over all N=2708 nodes; the host
concatenates the 8 [N, 64] slices into [N, 512].

Math: with s_ij = es_i + ed_j,
  exp(leakyrelu(s)) = max(exp(s), exp(0.2 s))          (exp is monotone)
and softmax over j is invariant to any per-i scale, so dividing by
exp(es_i) gives
  E_ij = adj_ji * max(G_i * D_j, B_j)
with G = exp(-0.8 es), B = exp(ed), D = exp(0.2 ed).

Engine plan (all rates HW-measured):
  U = max(G*D_j, B_j) is built per j-block [128 j, i free] by two engines:
  the Scalar engine computes final U for cols [0,S1) via two chained
  ACTIVATEs (relu(D*G - B) then +B, 0.85ns/col each) and the DVE builds
  [S1,NI) with one dual-op tensor_scalar (mult,max at 2x, 0.52ns/col).
  The adjacency mask is stored as 0xFFFF/0x0000 uint16 and applied as a
  bitwise AND on uint32 lanes: DVE 1x uint32 == bf16-mult 2x for cols
  [0,GM0), gpsimd (int ALU) for [GM0,NI).  adj/x DMAs are packed so each
  partition row carries >=10KB contiguous (DMA engines are packet-issue
  limited); xT is delivered column-group-major so each projection PSUM
  group finishes as its columns land and the conditioner pipelines under
  the matmul stream.  FiLM is applied by Scalar directly from PSUM.  The
  output matmul is "flipped": lhsT = E slice, rhs = [h | 1], accumulating
  po[i, 0:65] in PSUM across j blocks -- the result lands in [node,
  feature] layout (col 64 = softmax row sum).  es/ed fold into the
  projection matmul via host-precomputed W_m @ a_src / a_dst columns.
"""

import numpy as np
import ml_dtypes

N = 2708
INS = 1433
OUTS = 64
M = 8
HID = 64

FROWS = INS + 1      # 1434 features + ones row (bias folding)
FCH = 12             # 11 full 128-row chunks + one 26-row chunk
KROWS = [128] * 11 + [FROWS - 11 * 128]
NB = 22              # node blocks of 128
NJP = 11             # j-block pairs
NPAD = NB * 128      # 2816
NI = 2708            # valid i columns (trimmed)
RHSW = 130           # 64 W_m | 64 Wc1 | p_src | p_dst
# E-build column split: [0,S1) scalar 2-pass U (two chained ACTIVATEs,
# ~900ns fixed + 0.53ns/col each); [S1,NI) DVE dual-TS U.  The mask
# multiply runs on DVE only (gpsimd shares DVE's SBUF port and starves
# its 2-port ops) and is merged across each j-block pair to halve op
# overhead.
S1 = 992
NGRP = 6             # projection column groups (4,4,4,4,4,2 blocks)
GW = 512             # padded cols per group in xt SBUF layout
PGRP = [range(0, 4), range(4, 8), range(8, 12), range(12, 16),
        range(16, 20), range(20, 22)]

_CACHE = {}


def _build_nc():
    import concourse.bass as bass
    import concourse.mybir as mybir
    import concourse.tile as tile
    from concourse.masks import make_identity

    f32 = mybir.dt.float32
    bf16 = mybir.dt.bfloat16
    u16 = mybir.dt.uint16
    u32 = mybir.dt.uint32
    Alu = mybir.AluOpType
    Act = mybir.ActivationFunctionType

    nc = bass.Bass("TRN2", use_seq_codegen=True)

    # xQ[p, g, k, c] = xT[128k+p, XGO[g]+c]  (group-major, 12KB runs)
    xQ = nc.dram_tensor("xQ", [128, NGRP, FCH, GW], bf16,
                        kind="ExternalInput")
    WWd = nc.dram_tensor("WWd", [FROWS, RHSW], bf16, kind="ExternalInput")
    adjP = nc.dram_tensor("adjP", [NJP, 128, 2, NI], bf16,
                          kind="ExternalInput")
    wgb = nc.dram_tensor("wgb", [128, 2, OUTS], f32, kind="ExternalInput")
    cst = nc.dram_tensor("cst", [128, 4], f32, kind="ExternalInput")
    out = nc.dram_tensor("out", [128, NB, OUTS], f32, kind="ExternalOutput")

    with tile.TileContext(nc) as tc:
        with (
            tc.tile_pool(name="cpool", bufs=1) as cp,
            tc.tile_pool(name="big", bufs=1) as bigp,
            tc.tile_pool(name="hidp", bufs=2) as hp,
            tc.tile_pool(name="tmp", bufs=2) as tp,
            tc.tile_pool(name="maskp", bufs=4) as mp,
            tc.tile_pool(name="rp", bufs=2) as rp,
            tc.tile_pool(name="uep", bufs=2) as uep,
            tc.tile_pool(name="utp", bufs=2) as utp,
            tc.tile_pool(name="dram", bufs=1, space="DRAM") as dp,
        ):
            # ---- constants ----
            wgb_s = cp.tile([128, 2, OUTS], f32)
            nc.sync.dma_start(wgb_s, wgb[:])
            cst_s = cp.tile([128, 4], f32)
            nc.sync.dma_start(cst_s, cst[:])
            ident = cp.tile([128, 128], f32)
            make_identity(nc, ident)

            # ---- resident data (small WW first so matmuls can start on the
            # first xQ group) ----
            ww_all = bigp.tile([128, FCH, RHSW], bf16)
            nc.sync.dma_start(
                ww_all[:, 0:2, :],
                WWd[0:256].rearrange("(k p) w -> p k w", p=128))
            # xT column-group-major: all 12 K-chunks for node cols of group
            # g in one DMA with 12KB contiguous runs on both sides.  Group 0
            # is split by chunk pairs so its first matmuls start ~6us
            # earlier instead of waiting for the whole 1.6MB group.
            xt_all = bigp.tile([128, NGRP, FCH, GW], bf16)
            for c in range(FCH // 2):
                nc.sync.dma_start(xt_all[:, 0, 2 * c:2 * c + 2, :],
                                  xQ[:, 0, 2 * c:2 * c + 2, :])
            nc.sync.dma_start(
                ww_all[:, 2:11, :],
                WWd[256:1408].rearrange("(k p) w -> p k w", p=128))
            nc.sync.dma_start(ww_all[0:KROWS[11], 11, :], WWd[1408:FROWS, :])
            for g in range(1, NGRP):
                nc.sync.dma_start(xt_all[:, g], xQ[:, g])
            # adjacency prefetch: issue the first 4 pair DMAs NOW so the
            # sync engine isn't stuck behind the G-broadcast wait later.
            mts = {}
            for bp in range(NJP):
                if bp < 4:
                    mts[bp] = mp.tile([128, 2, NI], bf16, name=f"mt{bp}",
                                      tag="mt")
                    nc.sync.dma_start(mts[bp], adjP[bp])

            h_ext = bigp.tile([128, NB, OUTS + 1], bf16)
            hid_all = bigp.tile([128, NB, HID], f32)
            pq_all = bigp.tile([128, NB, 2], f32)
            gb_all = bigp.tile([128, NB, 2], f32)
            ee_all = bigp.tile([128, NB, 2], f32)
            BD = bigp.tile([128, 2, NB], f32)
            BDn = bigp.tile([128, NB], f32)
            g_all = bigp.tile([128, NPAD], bf16)
            Gn = bigp.tile([128, NB], f32)
            rinv = bigp.tile([128, NB], f32)
            os_all = bigp.tile([128, NB, OUTS], f32)

            nc.gpsimd.memset(h_ext[:, :, OUTS:OUTS + 1], 1.0)

            # ---- projection: [h0 | hidden | p,q] = x @ [W_m | Wc1 | Wa] ----
            # 4-block groups, 2 PSUM bank-sets; group g's columns arrive in
            # DMA order so its conditioner work pipelines under group g+1's
            # matmuls.  FiLM reads h0 straight out of PSUM.
            with tc.tile_pool(name="ppsum", bufs=2, space="PSUM") as pp:
                for gi, blocks in enumerate(PGRP):
                    nbk = blocks.stop - blocks.start
                    hhs = {}
                    for t, b in enumerate(blocks):
                        hhs[b] = pp.tile([128, RHSW], f32, name=f"hh{t}",
                                         tag=f"hh{t}")
                    for k in range(FCH):
                        kr = KROWS[k]
                        for t, b in enumerate(blocks):
                            nc.tensor.matmul(
                                hhs[b],
                                lhsT=xt_all[0:kr, gi, k,
                                            128 * t:128 * (t + 1)],
                                rhs=ww_all[0:kr, k, :],
                                start=(k == 0),
                                stop=(k == FCH - 1),
                            )
                    # drain PSUM: relu for the conditioner, pq copy
                    for t, b in enumerate(blocks):
                        nc.scalar.activation(hid_all[:, b, :],
                                             hhs[b][:, OUTS:128], Act.Relu)
                        nc.vector.tensor_copy(pq_all[:, b, :],
                                              hhs[b][:, 128:130])
                    b0, b1 = blocks.start, blocks.stop
                    # gamma/beta = hid @ Wc2 cols (+ bc2)
                    sh4g = (128, nbk, 2, OUTS)
                    scr = hp.tile([128, 4, 2, OUTS], f32, tag="scr")
                    nc.vector.tensor_tensor(
                        scr[:, 0:nbk],
                        hid_all[:, b0:b1].unsqueeze(2).to_broadcast(sh4g),
                        wgb_s.unsqueeze(1).to_broadcast(sh4g), Alu.mult)
                    nc.vector.tensor_reduce(
                        gb_all[:, b0:b1], scr[:, 0:nbk],
                        axis=mybir.AxisListType.X, op=Alu.add)
                    sh3 = (128, nbk, 2)
                    nc.vector.tensor_tensor(
                        gb_all[:, b0:b1], gb_all[:, b0:b1],
                        cst_s[:, 0:2].unsqueeze(1).to_broadcast(sh3), Alu.add)
                    # es/ed = gamma*(p,q) + beta*(sum a)
                    t1 = tp.tile([128, 4, 2], f32, tag="t1")
                    nc.vector.tensor_tensor(
                        t1[:, 0:nbk], pq_all[:, b0:b1],
                        gb_all[:, b0:b1, 0:1].to_broadcast(sh3), Alu.mult)
                    t2 = tp.tile([128, 4, 2], f32, tag="t2")
                    nc.vector.tensor_tensor(
                        t2[:, 0:nbk],
                        gb_all[:, b0:b1, 1:2].to_broadcast(sh3),
                        cst_s[:, 2:4].unsqueeze(1).to_broadcast(sh3), Alu.mult)
                    nc.vector.tensor_tensor(ee_all[:, b0:b1], t1[:, 0:nbk],
                                            t2[:, 0:nbk], Alu.add)
                    # FiLM: h = gamma * h0 + beta, straight from PSUM
                    for t, b in enumerate(blocks):
                        nc.scalar.activation(
                            h_ext[:, b, 0:OUTS], hhs[b][:, 0:OUTS],
                            Act.Identity, bias=gb_all[:, b, 1:2],
                            scale=gb_all[:, b, 0:1])

                # ---- per-j scalars (ready before the G transposes) ----
                nc.scalar.activation(BD[:, 0, :], ee_all[:, :, 1], Act.Exp)
                nc.scalar.activation(BD[:, 1, :], ee_all[:, :, 1], Act.Exp,
                                     scale=0.2)
                nc.vector.tensor_scalar_mul(BDn, BD[:, 0, :], -1.0)
                nc.scalar.activation(Gn, ee_all[:, :, 0], Act.Exp, scale=-0.8)

            # ---- G broadcast via PE transposes (all on-chip): for each
            # node block b, GnB = Gn[:, b] replicated along the free dim,
            # and transpose(GnB)[r, c] = G_{128b+c} for every partition r.
            with tc.tile_pool(name="gpsum", bufs=4, space="PSUM") as gp2:
                for b in range(NB):
                    GnB = tp.tile([128, 128], f32, name=f"gnb{b}", tag="gnb")
                    nc.vector.tensor_copy(
                        GnB, Gn[:, b:b + 1].to_broadcast((128, 128)))
                    gt = gp2.tile([128, 128], f32, name=f"gt{b}", tag="gt")
                    nc.tensor.transpose(gt, GnB, ident)
                    eng = nc.scalar if b % 2 else nc.vector
                    if b % 2:
                        nc.scalar.copy(g_all[:, 128 * b:128 * (b + 1)], gt)
                    else:
                        nc.vector.tensor_copy(
                            g_all[:, 128 * b:128 * (b + 1)], gt)

            # ---- attention: po[i, 0:64 | 64] += E_j^T @ [h_j | 1] ----
            # U built by scalar (cols [0,S1), two chained ACTIVATEs) and
            # DVE (dual-TS, [S1,NI)); mask applied as uint32 bitwise AND
            # split DVE / gpsimd.
            with tc.tile_pool(name="apsum", bufs=1, space="PSUM") as app:
                po = app.tile([128, NB, 128], f32, name="po", tag="po")
                for bp in range(NJP):
                    if bp in mts:
                        mt = mts[bp]
                    else:
                        mt = mp.tile([128, 2, NI], bf16, tag="mt")
                        nc.sync.dma_start(mt, adjP[bp])
                    U2 = utp.tile([128, 2, NI], bf16, tag="U2")
                    for q in range(2):
                        j = 2 * bp + q
                        R = rp.tile([128, S1], bf16, tag="R")
                        nc.scalar.activation(
                            R, g_all[:, 0:S1], Act.Relu,
                            bias=BDn[:, j:j + 1], scale=BD[:, 1, j:j + 1])
                        nc.scalar.activation(
                            U2[:, q, 0:S1], R, Act.Identity,
                            bias=BD[:, 0, j:j + 1])
                        nc.vector.tensor_scalar(
                            U2[:, q, S1:NI], g_all[:, S1:NI],
                            BD[:, 1, j:j + 1], BD[:, 0, j:j + 1],
                            Alu.mult, Alu.max)
                    E2 = uep.tile([128, 2, NI], bf16, tag="E2")
                    if bp == NJP - 1:
                        # split so the last pair's matmuls drain earlier
                        nc.vector.tensor_tensor(E2[:, 0], U2[:, 0],
                                                mt[:, 0], Alu.mult)
                        nc.vector.tensor_tensor(E2[:, 1], U2[:, 1],
                                                mt[:, 1], Alu.mult)
                    else:
                        nc.vector.tensor_tensor(E2, U2, mt, Alu.mult)
                    # start=True zeroes the whole 2KB PSUM bank (4 slices),
                    # so only the first matmul touching each bank may set it.
                    for q in range(2):
                        j = 2 * bp + q
                        for t in range(NB):
                            lo = 128 * t
                            hi = min(128 * (t + 1), NI)
                            nc.tensor.matmul(
                                po[0:hi - lo, t, 0:OUTS + 1],
                                lhsT=E2[:, q, lo:hi],
                                rhs=h_ext[:, j, :],
                                start=(j == 0 and t % 4 == 0),
                                stop=(j == NB - 1),
                                skip_group_check=True,
                            )

                # ---- normalize + store (already in [node, feature] layout) --
                nc.vector.reciprocal(rinv[:, 0:NB - 1],
                                     po[:, 0:NB - 1, OUTS])
                nc.vector.reciprocal(rinv[0:20, NB - 1:NB],
                                     po[0:20, NB - 1:NB, OUTS])
                for c in range(4):
                    b0 = 6 * c
                    b1 = min(6 * c + 6, NB)
                    shc = (128, b1 - b0, OUTS)
                    nc.vector.tensor_tensor(
                        os_all[:, b0:b1], po[:, b0:b1, 0:OUTS],
                        rinv[:, b0:b1].unsqueeze(2).to_broadcast(shc),
                        Alu.mult)
                    nc.sync.dma_start(out[:, b0:b1], os_all[:, b0:b1])

    nc.finalize()
    _split_multi_waits(nc, mybir)
    return nc


def _split_multi_waits(nc, mybir):
    """This toolchain's walrus accepts at most one sync wait per HW-decoded
    instruction; hoist extra waits onto standalone EventSemaphore ops on the
    same engine (engines execute their stream in order, so semantics hold)."""
    uid = [0]
    for f in nc.m.functions:
        for bb in f.blocks:
            insts = list(bb.instructions)
            out = []
            changed = False
            for ins in insts:
                si = ins.sync_info
                waits = list(si.on_wait) if si is not None and si.on_wait else []
                if len(waits) > 1:
                    changed = True
                    for w in waits[:-1]:
                        uid[0] += 1
                        ev = mybir.InstEventSemaphore(
                            name=f"splitw_{uid[0]}", ins=[], outs=[])
                        ev.engine = ins.engine
                        ev.sync_info = mybir.SyncInfo(on_wait=[w], on_update=[])
                        out.append(ev)
                    si.on_wait = [waits[-1]]
                out.append(ins)
            if changed:
                bb.instructions = out


def _prep_in_maps(x, adj, W, a_src, a_dst, Wc1, bc1, Wc2, bc2):
    bf = ml_dtypes.bfloat16
    # xT rows padded to 12*128, delivered column-group-major:
    # xQ[p, g, k, c] = xT[128k+p, 512g+c]
    xT_h = np.zeros((FCH * 128, NPAD + (NGRP * GW - NPAD)), dtype=bf)
    xT_h[:INS, :N] = x.T.astype(bf)
    xT_h[INS, :N] = 1.0  # ones row folds biases into the matmul
    xQ_h = np.ascontiguousarray(
        xT_h.reshape(FCH, 128, NGRP, GW).transpose(1, 2, 0, 3))

    # adjacency transposed, packed in j-block pairs, trimmed to NI cols:
    # adjP[bp, p, q, i] = adj[i, 128*(2*bp+q) + p]
    adjT_h = np.zeros((NPAD, NI), dtype=bf)
    adjT_h[:N, :N] = adj.T.astype(bf)
    adjP_h = np.ascontiguousarray(
        adjT_h.reshape(NJP, 2, 128, NI).transpose(0, 2, 1, 3))

    in_maps = []
    for m in range(M):
        WW_h = np.zeros((FROWS, RHSW), dtype=bf)
        WW_h[:INS, 0:OUTS] = W[m].astype(bf)
        WW_h[:INS, OUTS:128] = Wc1.astype(bf)
        WW_h[INS, OUTS:128] = bc1.astype(bf)
        WW_h[:INS, 128] = (W[m].astype(np.float64) @
                           a_src[m].astype(np.float64)).astype(bf)
        WW_h[:INS, 129] = (W[m].astype(np.float64) @
                           a_dst[m].astype(np.float64)).astype(bf)

        wgb_h = np.empty((128, 2, OUTS), dtype=np.float32)
        wgb_h[:, 0, :] = Wc2[:, m][None, :]
        wgb_h[:, 1, :] = Wc2[:, M + m][None, :]

        cst_h = np.empty((128, 4), dtype=np.float32)
        cst_h[:, 0] = bc2[m]
        cst_h[:, 1] = bc2[M + m]
        cst_h[:, 2] = float(np.sum(a_src[m], dtype=np.float64))
        cst_h[:, 3] = float(np.sum(a_dst[m], dtype=np.float64))

        in_maps.append({
            "xQ": xQ_h, "WWd": WW_h, "adjP": adjP_h,
            "wgb": wgb_h, "cst": cst_h,
        })
    return in_maps


def kernel(x, adj, W, a_src, a_dst, Wc1, bc1, Wc2, bc2, _profile=False):
    x = np.asarray(x, dtype=np.float32)
    adj = np.asarray(adj)
    W = np.asarray(W, dtype=np.float32)
    a_src = np.asarray(a_src, dtype=np.float32)
    a_dst = np.asarray(a_dst, dtype=np.float32)
    Wc1 = np.asarray(Wc1, dtype=np.float32)
    bc1 = np.asarray(bc1, dtype=np.float32)
    Wc2 = np.asarray(Wc2, dtype=np.float32)
    bc2 = np.asarray(bc2, dtype=np.float32)

    if "nc" not in _CACHE:
        _CACHE["nc"] = _build_nc()
    nc = _CACHE["nc"]

    from concourse.bass_utils import run_bass_kernel_spmd

    in_maps = _prep_in_maps(x, adj, W, a_src, a_dst, Wc1, bc1, Wc2, bc2)
    res = run_bass_kernel_spmd(
        nc, in_maps, core_ids=list(range(M)), trace=_profile,
    )
    full = np.empty((N, M * OUTS), dtype=np.float32)
    for m in range(M):
        o = res.results[m]["out"]  # [128, NB, OUTS], node = 128*t + p
        full[:, OUTS * m:OUTS * (m + 1)] = (
            o.transpose(1, 0, 2).reshape(NB * 128, OUTS)[0:N])
    if _profile:
        return full, res
    return full
